# revision 22
# baseline (speedup 1.0000x reference)
"""Distributed Trainium2 Bass kernel for multi-head attention.

Problem: b=2, n=2048, dim=1024, heads=16, head_dim=64 (inner=1024), f32 I/O.

Sharding (Megatron-style, per the hint): data-parallel over batch (cores 0-3
handle batch 0, cores 4-7 batch 1) x tensor-parallel over heads (core c%4
owns heads 4*(c%4)..4*(c%4)+3 via column shards of Wq/Wk/Wv and row shards
of Wo). Each core produces a partial [n, dim] output (its 4 heads pushed
through its Wo row block); the unshard step sums the 4 partials per batch
(the "all-reduce after to_out" done at gather time -- measured on this fleet,
the on-device collective is ~60us/MB which would dominate the compute).

Per-core device pipeline (all matmuls bf16, f32 PSUM accumulation):
  1. qpT/kpT = Wq^T q^T etc in transposed [inner_loc, n] layout, built
     per 512-column chunk; vp in natural [n, inner_loc] layout padded with
     a ones column per head (so P@V also yields the softmax denominator
     for free as row 64).
  2. S^T = kh qh^T per head in [n_k, n_q] layout (row-packed pairwise in
     the PE array); exp on ScalarE with the 1/sqrt(dh) scale folded in.
  3. O^T (+denominator row) accumulated in PSUM over n_k tiles.
  4. Per n_q chunk: reciprocal of denominators, broadcast via a tiny
     mask-matmul, normalize O^T tiles, then the Wo projection emits the
     final [n_q, dim] rows (bf16 partials; host sums in f32).

v2 scheduling: inputs land via ~16 BATCHED column-chunk DMAs (1 MB each)
instead of 64 per-tile DMAs -- the per-dma_start issue cost (~650 ns of
queue-sequencer time) made the v1 input stream issue-paced (~45 us); now
it is transfer-paced and the first exp fires ~15 us in. kT chunks ride the
sync queue, qT + weights the gpsimd queue, vT the vector queue, with the
late chunks issued from inside the unit loop so head-critical bytes get
the HBM bandwidth first. The u1 feeder interleaves qproj/vproj so next-unit
inputs complete several batches before their consumers. The tail skips the
pair-7 ot assembly: reciprocals read the PSUM denominator rows directly,
K=1 ones-matmuls broadcast them, and the chunk-3 Wo runs as a 3-matmul
split-K group (pair-0 ot_sb K=128 + two K=64 stage tiles), cutting the
post-last-exp chain from ~18 us to ~9 us.
"""

import sys

if "/opt/trn_rl_repo" not in sys.path:
    sys.path.insert(0, "/opt/trn_rl_repo")

import numpy as np
import ml_dtypes

import concourse.bass as bass
import concourse.mybir as mybir
from concourse import bacc, tile
from concourse.bass_utils import run_bass_kernel_spmd
from concourse.masks import make_identity

BF16 = mybir.dt.bfloat16
F32 = mybir.dt.float32
NPBF16 = ml_dtypes.bfloat16

B = 2
N = 2048          # sequence length (full, per batch)
D = 1024          # model dim
H = 16            # total heads
DH = 64           # head dim
H_LOC = 4         # heads per core
INNER = H_LOC * DH  # 256, local inner dim
KC = D // 128     # 8 contraction chunks over model dim
KT = N // 128     # 16 k-tiles over sequence
NQC = N // 512    # 4 query chunks of 512
SCALE = DH ** -0.5
ES_BUFS = 32      # es slot pool


def _build_nc():
    nc = bacc.Bacc("TRN2", target_bir_lowering=False, debug=False, num_devices=8)

    qT = nc.declare_dram_parameter("qT", [D, N], BF16, isOutput=False)
    kT = nc.declare_dram_parameter("kT", [D, N], BF16, isOutput=False)
    vT = nc.declare_dram_parameter("vT", [D, N], BF16, isOutput=False)
    wq = nc.declare_dram_parameter("wq", [D, INNER], BF16, isOutput=False)
    wk = nc.declare_dram_parameter("wk", [D, INNER], BF16, isOutput=False)
    wv = nc.declare_dram_parameter("wv", [D, INNER], BF16, isOutput=False)
    wo = nc.declare_dram_parameter("wo", [INNER, D], BF16, isOutput=False)
    emask = nc.declare_dram_parameter("emask", [4, 256], BF16, isOutput=False)
    out = nc.declare_dram_parameter("out", [N, D], BF16, isOutput=True)

    # batched-DMA source views: dram row (k*128 + p) -> partition p, k-tile k
    kT_v = kT.rearrange("(k p) n -> p k n", p=128)
    qT_v = qT.rearrange("(k p) n -> p k n", p=128)
    vT_v = vT.rearrange("(k p) n -> p k n", p=128)
    wq_v = wq.rearrange("(k p) i -> p k i", p=128)
    wk_v = wk.rearrange("(k p) i -> p k i", p=128)
    wv_v = wv.rearrange("(k p) i -> p k i", p=128)
    wo_v = wo.rearrange("(m p) d -> p m d", p=128)

    with tile.TileContext(nc) as tc:
        with (
            tc.tile_pool(name="persist", bufs=1) as pp,
            tc.tile_pool(name="xkv", bufs=20) as xkv,
            tc.tile_pool(name="work", bufs=2) as wk_pool,
            tc.tile_pool(name="psum", bufs=2, space="PSUM") as psum,
        ):
            # ---- ScalarE exp table preload + PE clock warm-up burst
            warm = pp.tile([1, 16], F32, tag="warm", name="warm")
            nc.vector.memset(warm[:], 0.0)
            nc.scalar.activation(warm[:], warm[:], mybir.ActivationFunctionType.Exp)
            wa = pp.tile([128, 16], BF16, tag="wa", name="wa")
            wr = pp.tile([128, 448], BF16, tag="wr", name="wr")
            nc.vector.memset(wa[:], 0.0)
            nc.vector.memset(wr[:], 0.0)

            # ---- persistent weight tiles (one batched DMA each) ------------
            wk_t = pp.tile([128, KC * INNER], BF16, tag="wk", name="wk")
            wq_t = pp.tile([128, KC * INNER], BF16, tag="wq", name="wq")
            wv_t = pp.tile([128, KC * INNER], BF16, tag="wv", name="wv")
            wo_t = pp.tile([128, 2 * D], BF16, tag="wo", name="wo")
            wo_tail = pp.tile([64, D], BF16, tag="wotl", name="wotl")
            nc.sync.dma_start(
                wk_t[:].rearrange("p (k i) -> p k i", i=INNER), wk_v[:, :, :]
            )

            def w_sl(w_t, k, m):
                return w_t[:, INNER * k + 128 * m:INNER * k + 128 * (m + 1)]

            wo_sb = [wo_t[:, D * m:D * (m + 1)] for m in range(2)]

            # ---- input chunks: one [128, 8*512] tile per 512-col chunk -----
            k_chunk = [None] * NQC
            q_chunk = [None] * NQC
            v_chunk = [None] * NQC

            def dma_in(store, pool_tag, bufs, view, c, eng):
                t = xkv.tile([128, KC * 512], BF16, tag=pool_tag,
                             name=f"{pool_tag}{c}", bufs=bufs)
                eng.dma_start(
                    t[:].rearrange("p (k n) -> p k n", n=512),
                    view[:, :, 512 * c:512 * (c + 1)],
                )
                store[c] = t

            # head: one sync queue, priority order (same-queue transfers are
            # FIFO, so this is the only way to give the first-exp-critical
            # bytes the full HBM bandwidth): wk, kc0, wq, qc0, then the rest
            k0h = []
            for hh in range(2):
                t = xkv.tile([128, 4 * 512], BF16, tag="kh", name=f"kh{hh}", bufs=2)
                nc.sync.dma_start(
                    t[:].rearrange("p (k n) -> p k n", n=512),
                    kT_v[:, 4 * hh:4 * (hh + 1), 0:512],
                )
                k0h.append(t)
            nc.sync.dma_start(
                wq_t[:].rearrange("p (k i) -> p k i", i=INNER), wq_v[:, :, :]
            )
            q0h = []
            for hh in range(2):
                t = xkv.tile([128, 4 * 512], BF16, tag="qh", name=f"qh{hh}", bufs=2)
                nc.sync.dma_start(
                    t[:].rearrange("p (k n) -> p k n", n=512),
                    qT_v[:, 4 * hh:4 * (hh + 1), 0:512],
                )
                q0h.append(t)
            dma_in(k_chunk, "kc", 3, kT_v, 1, nc.sync)
            dma_in(k_chunk, "kc", 3, kT_v, 2, nc.sync)
            dma_in(q_chunk, "qc", 2, qT_v, 1, nc.sync)
            dma_in(k_chunk, "kc", 3, kT_v, 3, nc.sync)

            # PE warm-up bursts (bridge until kc0 lands)
            for i in range(28):
                wps = psum.tile([16, 448], F32, tag="epi", name="wps", bufs=2)
                nc.tensor.matmul(wps[:], lhsT=wa[:], rhs=wr[:], start=True, stop=True)

            # ---- identity for vpa transposes + ones for den broadcast
            ident = pp.tile([128, 128], BF16, tag="ident", name="ident")
            make_identity(nc, ident[:])
            ones65 = pp.tile([65, 64], BF16, tag="ones65", name="ones65")
            nc.vector.memset(ones65[:], 1.0)

            # ---- projection outputs, one tile per (m, chunk) -----------------
            kp_sb = [[pp.tile([128, 512], BF16, tag=f"kp{m}{c}", name=f"kp{m}{c}")
                      for c in range(NQC)] for m in range(2)]
            qp_sb = [[pp.tile([128, 512], BF16, tag=f"qp{m}{c}", name=f"qp{m}{c}")
                      for c in range(NQC)] for m in range(2)]
            vpt_sb = [[pp.tile([128, 512], BF16, tag=f"vp{m}{c}", name=f"vp{m}{c}")
                       for c in range(NQC)] for m in range(2)]
            vpa = [pp.tile([128, H_LOC * 65], BF16, tag=f"vpa{j}", name=f"vpa{j}") for j in range(KT)]

            def gen_proj_chunk(w_t, src_of, dst, m, c):
                """One 512-col projection chunk: 8 accumulating matmuls + copy.
                Yields after each matmul for interleaving."""
                ps = psum.tile([128, 512], F32, tag="epi", name="pps", bufs=2)
                for k in range(KC):
                    nc.tensor.matmul(
                        ps[:],
                        lhsT=w_sl(w_t, k, m),
                        rhs=src_of(k, c),
                        start=(k == 0),
                        stop=(k == KC - 1),
                    )
                    yield
                nc.vector.tensor_copy(dst[:], ps[:])

            def k_src(k, c):
                if c == 0:
                    return k0h[k // 4][:, 512 * (k % 4):512 * (k % 4 + 1)]
                return k_chunk[c][:, 512 * k:512 * (k + 1)]

            def v_src(k, c):
                return v_chunk[c][:, 512 * k:512 * (k + 1)]

            def q_src(k, c):
                if c == 0:
                    return q0h[k // 4][:, 512 * (k % 4):512 * (k % 4 + 1)]
                return q_chunk[c][:, 512 * k:512 * (k + 1)]

            def gen_kproj(m, c):
                return gen_proj_chunk(wk_t, k_src, kp_sb[m][c], m, c)

            def gen_qproj(m, c):
                return gen_proj_chunk(wq_t, q_src, qp_sb[m][c], m, c)

            def gen_vproj(m, c):
                return gen_proj_chunk(wv_t, v_src, vpt_sb[m][c], m, c)

            def chain(*gens):
                for g in gens:
                    for _ in g:
                        yield

            def emit_s_exp(m, c, j):
                """S^T + exp for k-tile j, both heads of pair m (row-packed)."""
                sp = psum.tile([128, 1024], F32, tag="sp", name="sp", bufs=2)
                es = wk_pool.tile([128, 1024], BF16, tag="es", name="es", bufs=ES_BUFS)
                for h in range(2):
                    p0 = 64 * h
                    nc.tensor.matmul(
                        sp[:, 512 * h:512 * (h + 1)],
                        lhsT=kp_sb[m][j // 4][p0:p0 + 64, 128 * (j % 4):128 * (j % 4 + 1)],
                        rhs=qp_sb[m][c][p0:p0 + 64, :],
                        start=True,
                        stop=True,
                    )
                nc.scalar.activation(
                    es[:], sp[:], mybir.ActivationFunctionType.Exp,
                )
                return es

            def emit_o(m, j, es, ot_ps):
                for h in range(2):
                    hl = 2 * m + h
                    nc.tensor.matmul(
                        ot_ps[h][:],
                        lhsT=vpa[j][:, 65 * hl:65 * hl + 65],
                        rhs=es[:, 512 * h:512 * (h + 1)],
                        start=(j == 0),
                        stop=(j == KT - 1),
                    )

            def emit_vpa(j, m):
                tp = psum.tile([128, 128], BF16, tag="epi", name="tp", bufs=2)
                nc.tensor.transpose(
                    tp[:], vpt_sb[m][j // 4][:, 128 * (j % 4):128 * (j % 4 + 1)],
                    ident[:],
                )
                dst = vpa[j][:, 130 * m:130 * (m + 1)].rearrange(
                    "p (h e) -> p h e", e=65
                )[:, :, 0:64]
                nc.vector.tensor_copy(dst, tp[:].rearrange("p (h e) -> p h e", e=64))

            def new_ot_ps():
                return [
                    psum.tile([65, 512], F32, tag="otps", name=f"otps{h}", bufs=2)
                    for h in range(2)
                ]

            def unload_pair(m, ot_ps, pair_tile, tail=False):
                # one 65-row copy per head (O rows + bf16 denominator row);
                # the den rows stay in the stage tiles (row 64) and are
                # broadcast straight from partition 64 by the norm matmul
                stage_e = wk_pool.tile([65, 512], BF16, tag="stge", name="stge", bufs=4)
                stage_o = wk_pool.tile([65, 512], BF16, tag="stgo", name="stgo", bufs=4)
                nc.scalar.copy(stage_e[:], ot_ps[0][:])
                nc.vector.tensor_copy(stage_o[:], ot_ps[1][:])
                nc.vector.tensor_copy(pair_tile[0:64, :], stage_e[0:64, :])
                nc.sync.dma_start(pair_tile[64:128, :], stage_o[0:64, :])
                return stage_e, stage_o

            def emit_epi_norm(ot_sb_m, stage_pair, m):
                """normalize one pair: broadcast the raw den rows (partition
                64 of the stage tiles) into a [128,512] psum via two quadrant
                K=1 ones-matmuls, reciprocal straight off PSUM, multiply."""
                stage_e, stage_o = stage_pair
                bc = psum.tile([128, 512], F32, tag="epi", name="bc", bufs=2)
                nc.tensor.matmul(
                    bc[0:64, :], lhsT=ones65[64:65, 0:64], rhs=stage_e[64:65, :],
                    start=True, stop=True, tile_position=(64, 0),
                )
                nc.tensor.matmul(
                    bc[64:128, :], lhsT=ones65[64:65, 0:64], rhs=stage_o[64:65, :],
                    start=True, stop=True, tile_position=(64, 64),
                )
                recip_bc = wk_pool.tile([128, 512], F32, tag="recbc", name="recbc", bufs=2)
                nc.vector.reciprocal_approx_fast(recip_bc[:], bc[:])
                nc.vector.tensor_mul(ot_sb_m[:], ot_sb_m[:], recip_bc[:])

            def gen_epi_wo(c, ot_sb, tail=False):
                """the Wo projection for one n_q chunk; yields after matmuls."""
                for s in range(4):
                    for dch in range(2):
                        ops = psum.tile([128, 512], F32, tag="epi", name="op", bufs=2)
                        for m in range(2):
                            nc.tensor.matmul(
                                ops[:],
                                lhsT=ot_sb[m][:, 128 * s:128 * (s + 1)],
                                rhs=wo_sb[m][:, 512 * dch:512 * (dch + 1)],
                                start=(m == 0),
                                stop=(m == 1),
                            )
                            yield
                        o_sb = wk_pool.tile([128, 512], BF16, tag="osb", name="osb", bufs=4)
                        if tail and (s + dch) % 2 == 0:
                            nc.scalar.copy(o_sb[:], ops[:])
                        else:
                            nc.vector.tensor_copy(o_sb[:], ops[:])
                        r0 = 512 * c + 128 * s
                        nc.sync.dma_start(
                            out[r0:r0 + 128, 512 * dch:512 * (dch + 1)], o_sb[:]
                        )

            def gen_epilogue(c, ot_sb, stage_pairs, tail=False):
                for m in range(2):
                    emit_epi_norm(ot_sb[m], stage_pairs[m], m)
                    yield
                for _ in gen_epi_wo(c, ot_sb, tail):
                    yield

            # ---- emission schedule ------------------------------------------
            # unit u = (pair m=u%2, chunk c=u//2), 16 S+exp batches each
            # (batch b covers k-tile j=b for both heads). O-passes, input
            # projections, vpa transposes and epilogues ride in each unit's
            # feeder slots; the last two units' O-passes run split-half so
            # only one PSUM accumulator pair is alive at a time.
            units = [(u % 2, u // 2) for u in range(2 * NQC)]
            es_held = {}
            ot_ps_of = {}
            ot_sb_of = {}
            stage_of = {}
            epi_gen = [None]
            _DONE = object()

            def start_epi(cp, tail=False):
                epi_gen[0] = gen_epilogue(
                    cp, [ot_sb_of[2 * cp], ot_sb_of[2 * cp + 1]],
                    [stage_of[(cp, 0)], stage_of[(cp, 1)]], tail=tail,
                )

            def epi_step(nmax):
                g = epi_gen[0]
                if g is None:
                    return
                for _ in range(nmax):
                    if next(g, _DONE) is _DONE:
                        epi_gen[0] = None
                        return

            def epi_drain():
                if epi_gen[0] is not None:
                    for _ in epi_gen[0]:
                        pass
                    epi_gen[0] = None

            # prologue: first projection chunks (kp m0 c0, qp m0 c0)
            for _ in chain(gen_kproj(0, 0), gen_qproj(0, 0)):
                pass

            feeders = {
                0: chain(gen_kproj(0, 1), gen_kproj(1, 0), gen_qproj(1, 0),
                         gen_kproj(0, 2), gen_kproj(0, 3), gen_kproj(1, 1),
                         gen_kproj(1, 2), gen_kproj(1, 3)),           # 64
                1: chain(gen_vproj(0, 0), gen_vproj(1, 0), gen_qproj(0, 1),
                         gen_vproj(0, 1), gen_vproj(1, 1), gen_qproj(1, 1),
                         gen_vproj(0, 2), gen_vproj(1, 2),
                         gen_vproj(0, 3), gen_vproj(1, 3)),           # 80
                2: chain(gen_qproj(0, 2), gen_qproj(1, 2)),           # 16
                3: chain(gen_qproj(0, 3), gen_qproj(1, 3)),           # 16
            }
            fslots = {0: 4, 1: 5, 2: 1, 3: 1}

            for u, (m, c) in enumerate(units):
                ot_sb_of[u] = wk_pool.tile(
                    [128, 512], BF16, tag="ot", name=f"ot{u}", bufs=4
                )
                feeder = feeders.get(u)
                if u == 0:
                    for j in range(KT):
                        nc.vector.memset(vpa[j][:], 1.0)
                if 2 <= u <= 4:
                    ot_ps_of[u - 2] = new_ot_ps()
                es_held[u] = []

                for b in range(KT):
                    # late input DMA issues (keep head bandwidth for k/q c0)
                    if u == 0 and b == 2:
                        dma_in(v_chunk, "vc", 2, vT_v, 0, nc.sync)
                        nc.sync.dma_start(
                            wv_t[:].rearrange("p (k i) -> p k i", i=INNER),
                            wv_v[:, :, :],
                        )
                        dma_in(v_chunk, "vc", 2, vT_v, 1, nc.sync)
                    if u == 1 and b == 0:
                        dma_in(v_chunk, "vc", 2, vT_v, 2, nc.sync)
                        dma_in(q_chunk, "qc", 2, qT_v, 2, nc.sync)
                    if u == 1 and b == 4:
                        dma_in(v_chunk, "vc", 2, vT_v, 3, nc.sync)
                        dma_in(q_chunk, "qc", 2, qT_v, 3, nc.sync)
                        nc.sync.dma_start(
                            wo_t[:].rearrange("p (m d) -> p m d", d=D),
                            wo_v[:, :, :],
                        )
                        nc.sync.dma_start(wo_tail[:], wo[192:256, :])

                    # S first in the final unit's second half: es[j=b] may be
                    # consumed by this unit's own O-pass in the same slot
                    s_first = (u == 7 and b >= 8)
                    if s_first:
                        es_held[u].append(emit_s_exp(m, c, b))
                    if feeder is not None:
                        for _ in range(fslots[u]):
                            next(feeder, None)
                    # O-pass / epilogue work for this slot
                    if 2 <= u <= 4:
                        up = u - 2
                        if up == 0:
                            if b == 0:
                                emit_vpa(0, 0)
                                emit_vpa(0, 1)
                            if b < KT - 1:
                                emit_vpa(b + 1, 0)
                                emit_vpa(b + 1, 1)
                        emit_o(units[up][0], b, es_held[up][b], ot_ps_of[up])
                        if u == 4:
                            epi_step(2)       # chunk-0 epilogue
                    elif u == 5:
                        if b == 0:
                            ot_ps_of[3] = new_ot_ps()
                        if b < 8:
                            for j in (2 * b, 2 * b + 1):
                                emit_o(1, j, es_held[3][j], ot_ps_of[3])
                            epi_step(1)       # chunk-0 leftovers
                        else:
                            if b == 8:
                                stage_of[(1, 1)] = unload_pair(1, ot_ps_of[3], ot_sb_of[3])
                                del es_held[3]
                                epi_drain()
                                start_epi(1)  # chunk-1 epilogue
                                ot_ps_of[4] = new_ot_ps()
                            for j in (2 * (b - 8), 2 * (b - 8) + 1):
                                emit_o(0, j, es_held[4][j], ot_ps_of[4])
                            epi_step(2)
                    elif u == 6:
                        if b == 0:
                            ot_ps_of[5] = new_ot_ps()
                        emit_o(1, b, es_held[5][b], ot_ps_of[5])
                        epi_step(2)           # chunk-1 leftovers
                    elif u == 7:
                        if b == 0:
                            ot_ps_of[6] = new_ot_ps()
                        if b < 8:
                            for j in (2 * b, 2 * b + 1):
                                emit_o(0, j, es_held[6][j], ot_ps_of[6])
                            epi_step(1)       # chunk-2 epilogue
                        else:
                            if b == 8:
                                stage_of[(3, 0)] = unload_pair(0, ot_ps_of[6], ot_sb_of[6])
                                del es_held[6]
                                ot_ps_of[7] = new_ot_ps()
                                # normalize chunk-3 pair 0 now; pair 1 goes
                                # through the split-K tail path
                                emit_epi_norm(ot_sb_of[6], stage_of[(3, 0)], 0)
                            for j in (2 * (b - 8), 2 * (b - 8) + 1):
                                emit_o(1, j, es_held[7][j], ot_ps_of[7])
                            epi_step(2)
                    if not s_first:
                        es_held[u].append(emit_s_exp(m, c, b))

                if feeder is not None:
                    for _ in feeder:
                        pass
                if 2 <= u <= 4:
                    up = u - 2
                    mp, cp = units[up]
                    stage_of[(cp, mp)] = unload_pair(mp, ot_ps_of[up], ot_sb_of[up])
                    del es_held[up]
                    if mp == 1:
                        start_epi(cp)         # chunk-0 epilogue (spread in u4)
                if u == 5:
                    stage_of[(2, 0)] = unload_pair(0, ot_ps_of[4], ot_sb_of[4])
                    del es_held[4]
                if u == 6:
                    stage_of[(2, 1)] = unload_pair(1, ot_ps_of[5], ot_sb_of[5])
                    del es_held[5]
                    epi_drain()               # finish chunk-1 epilogue
                    start_epi(2)              # chunk-2 epilogue (spread in u7)

            # ---- tail: finish chunk-2 epilogue, then pair-7 without any
            # assembly or den DMA: stage copies, den broadcast straight from
            # partition 64, reciprocal off PSUM, in-place stage normalize,
            # and a 3-matmul split-K Wo per output block. A burst block after
            # the bc matmuls keeps the PE p-state up through the DVE chain.
            epi_drain()
            otp7 = ot_ps_of[7]
            stage_e = wk_pool.tile([65, 512], BF16, tag="stge", name="stge7", bufs=4)
            stage_o = wk_pool.tile([65, 512], BF16, tag="stgo", name="stgo7", bufs=4)
            nc.scalar.copy(stage_e[:], otp7[0][:])
            nc.vector.tensor_copy(stage_o[:], otp7[1][:])
            bc7 = psum.tile([64, 1024], F32, tag="sp", name="bc7", bufs=2)
            nc.tensor.matmul(
                bc7[:, 0:512], lhsT=ones65[64:65, 0:64], rhs=stage_e[64:65, :],
                start=True, stop=True, tile_position=(64, 0),
            )
            nc.tensor.matmul(
                bc7[:, 512:1024], lhsT=ones65[64:65, 0:64], rhs=stage_o[64:65, :],
                start=True, stop=True, tile_position=(64, 0),
            )
            for i in range(6):
                wps = psum.tile([16, 448], F32, tag="epi", name="wps", bufs=2)
                nc.tensor.matmul(wps[:], lhsT=wa[:], rhs=wr[:], start=True, stop=True)
            rec7e = wk_pool.tile([128, 512], F32, tag="recbc", name="rec7e", bufs=2)
            rec7o = wk_pool.tile([128, 512], F32, tag="recbc", name="rec7o", bufs=2)
            nc.vector.reciprocal_approx_fast(rec7e[0:64, :], bc7[:, 0:512])
            nc.vector.reciprocal_approx_fast(rec7o[0:64, :], bc7[:, 512:1024])
            nc.vector.tensor_mul(stage_e[0:64, :], stage_e[0:64, :], rec7e[0:64, :])
            nc.vector.tensor_mul(stage_o[0:64, :], stage_o[0:64, :], rec7o[0:64, :])

            for s in range(4):
                for dch in range(2):
                    ops = psum.tile([128, 512], F32, tag="epi", name="op7", bufs=2)
                    nc.tensor.matmul(
                        ops[:],
                        lhsT=ot_sb_of[6][:, 128 * s:128 * (s + 1)],
                        rhs=wo_sb[0][:, 512 * dch:512 * (dch + 1)],
                        start=True, stop=False,
                    )
                    nc.tensor.matmul(
                        ops[:],
                        lhsT=stage_e[0:64, 128 * s:128 * (s + 1)],
                        rhs=wo_sb[1][0:64, 512 * dch:512 * (dch + 1)],
                        start=False, stop=False,
                    )
                    nc.tensor.matmul(
                        ops[:],
                        lhsT=stage_o[0:64, 128 * s:128 * (s + 1)],
                        rhs=wo_tail[:, 512 * dch:512 * (dch + 1)],
                        start=False, stop=True,
                    )
                    o_sb = wk_pool.tile([128, 512], BF16, tag="osb", name="osb", bufs=4)
                    if (s + dch) % 2 == 0:
                        nc.scalar.copy(o_sb[:], ops[:])
                    else:
                        nc.vector.tensor_copy(o_sb[:], ops[:])
                    r0 = 512 * 3 + 128 * s
                    nc.sync.dma_start(
                        out[r0:r0 + 128, 512 * dch:512 * (dch + 1)], o_sb[:]
                    )

    nc.compile()
    return nc


_NC_CACHE = None


def _get_nc():
    global _NC_CACHE
    if _NC_CACHE is None:
        _NC_CACHE = _build_nc()
    return _NC_CACHE


def kernel(q, k, v, Wq, Wk, Wv, Wo):
    q = np.asarray(q, dtype=np.float32)
    k = np.asarray(k, dtype=np.float32)
    v = np.asarray(v, dtype=np.float32)
    Wq = np.asarray(Wq, dtype=np.float32)
    Wk = np.asarray(Wk, dtype=np.float32)
    Wv = np.asarray(Wv, dtype=np.float32)
    Wo = np.asarray(Wo, dtype=np.float32)

    Wq = Wq * np.float32(SCALE)  # fold the 1/sqrt(dh) softmax scale into Wq
    qT = [np.ascontiguousarray(q[g].T).astype(NPBF16) for g in range(B)]
    kT = [np.ascontiguousarray(k[g].T).astype(NPBF16) for g in range(B)]
    vT = [np.ascontiguousarray(v[g].T).astype(NPBF16) for g in range(B)]
    wq_b = Wq.astype(NPBF16)
    wk_b = Wk.astype(NPBF16)
    wv_b = Wv.astype(NPBF16)
    wo_b = Wo.astype(NPBF16)
    emask = np.zeros((4, 256), NPBF16)
    for m in range(2):
        emask[2 * m, 128 * m:128 * m + 64] = 1
        emask[2 * m + 1, 128 * m + 64:128 * m + 128] = 1

    in_maps = []
    for c in range(8):
        g, t = c // 4, c % 4
        sl = slice(INNER * t, INNER * (t + 1))
        in_maps.append({
            "qT": qT[g],
            "kT": kT[g],
            "vT": vT[g],
            "wq": np.ascontiguousarray(wq_b[:, sl]),
            "wk": np.ascontiguousarray(wk_b[:, sl]),
            "wv": np.ascontiguousarray(wv_b[:, sl]),
            "wo": np.ascontiguousarray(wo_b[sl, :]),
            "emask": emask,
        })

    nc = _get_nc()
    res = run_bass_kernel_spmd(nc, in_maps, core_ids=list(range(8)))

    out = np.empty((B, N, D), np.float32)
    for g in range(B):
        acc = res.results[4 * g]["out"].astype(np.float32)
        for t in range(1, 4):
            acc = acc + res.results[4 * g + t]["out"].astype(np.float32)
        out[g] = acc
    return out
